# revision 6
# baseline (speedup 1.0000x reference)
"""Coupled-attention module as a distributed Bass/Tile kernel on 8 TRN2 cores.

Math notes (exact algebra, not approximations):
- The differential-attention scores are constant along the softmax axis, so
  softmax yields exactly uniform 1/S weights: diff_vector collapses to the
  per-batch mean of (y @ dv_w + dv_b), broadcast over sequence. dq/dk are dead.
- Sharding: rows of the flattened (B*S, H) activations, 256 per core; cores
  0-3 own batch 0, 4-7 batch 1. Each core redundantly computes full-batch K/V
  (an AllGather reshard measures ~40us for 792KB here - slower than the
  ~25us of redundant PE work it would save).
- All activations live channel-major [C, rows] on chip, so weights feed the
  PE as natural [K, M] lhsT tiles, and the two sequence-axis softmaxes in the
  gating network reduce along the free dim. Their denominators are summed
  across the 4-core batch group with tiny AllReduces.
- All DRAM tensors are pre-packed on the host so every SBUF partition's data
  is one contiguous DRAM run (single DMA packet per partition).
- The four diff-branch matvec chains (m, theta1, bias1, bias2) run with the
  weight as the MOVING tensor (12 wide matmuls instead of 36 N=1 matmuls),
  then a 6-block PE transpose returns the row vector to partition-major.
- Compute in bf16 with fp32 accumulation (all GEMMs), exp/tanh/sigmoid on ACT.
  Attention exp is applied to two 128-key blocks at once ([128,512] ACT ops).
- Per-head softmax denominators invert via the single-op approx reciprocal
  (~18 valid bits) instead of the multi-op exact DVE reciprocal.
- The AllReduce-independent work (gamma1, z2a, voa GEMMs, bias2 matvec, the
  vanT half of the nf gate) is pre-accumulated while AllReduce-1 is in
  flight; the diff tail (dout, dfus, gate's dfus half) fills AllReduce-2.
"""

import numpy as np
import ml_dtypes

import concourse.bass as bass
import concourse.mybir as mybir
import concourse.tile as tile
from concourse import bacc
from concourse.bass_utils import run_bass_kernel_spmd

B, S, H = 2, 1024, 768
NH, DH = 12, 64
P = 128
RV = 256            # rows per core
KC = H // P         # 6 channel chunks
JC = S // P         # 8 sequence chunks
GROUPS = [[0, 1, 2, 3], [4, 5, 6, 7]]
SCALE = 1.0 / 8.0   # 1/sqrt(DH)

bf16 = mybir.dt.bfloat16
f32 = mybir.dt.float32
AF = mybir.ActivationFunctionType
ALU = mybir.AluOpType
nbf16 = ml_dtypes.bfloat16

W768 = ["vq_w", "vk_w", "vv_w", "dv_w", "WD_w", "van_fc_w", "WV_w", "diff_fc_w",
        "diff_fus_w", "van_fus_w", "nf_w", "final_w"]
W1536 = ["d_theta_w", "v_gamma_w", "diff_out_w", "van_out_w"]
BIAS = ["vq_b", "vk_b", "dv_b", "van_fc_b", "d_theta_b", "diff_fc_b",
        "v_gamma_b", "diff_out_b", "van_out_b", "diff_fus_b", "van_fus_b",
        "nf_b", "final_b"]


def build(has_vvb: bool):
    nc = bacc.Bacc(None, target_bir_lowering=False, debug=False, num_devices=8)

    xT_d = nc.dram_tensor("xT", [P, KC, RV], bf16, kind="ExternalInput")
    yT_d = nc.dram_tensor("yT", [P, KC, S], bf16, kind="ExternalInput")
    wd = {}
    for w in W768:
        wd[w] = nc.dram_tensor(w, [P, KC, H], bf16, kind="ExternalInput")
    for w in W1536:
        wd[w] = nc.dram_tensor(w, [2, P, KC, H], bf16, kind="ExternalInput")
    wd["gate_w"] = nc.dram_tensor("gate_w", [P, 2 * KC], bf16, kind="ExternalInput")
    wd["nf_out_w"] = nc.dram_tensor("nf_out_w", [P, 2 * KC], bf16,
                                    kind="ExternalInput")
    bd = {}
    for b in BIAS:
        bd[b] = nc.dram_tensor(b, [P, KC], f32, kind="ExternalInput")
    if has_vvb:
        bd["vv_b"] = nc.dram_tensor("vv_b", [P, KC], f32, kind="ExternalInput")
    out_d = nc.dram_tensor("outT", [P, KC, RV], f32, kind="ExternalOutput")

    with tile.TileContext(nc, num_cores=8) as tc:
        with (
            tc.tile_pool(name="wpool", bufs=5) as wp,
            tc.tile_pool(name="wsmall", bufs=2) as wsp,
            tc.tile_pool(name="acts", bufs=1) as ap,
            tc.tile_pool(name="loop", bufs=2) as lp,
            tc.tile_pool(name="psum", bufs=8, space="PSUM") as pp,
            tc.tile_pool(name="dram", bufs=4, space="DRAM") as dp,
        ):
            def wtile(name, half=None):
                t = wp.tile([P, KC, H], bf16, name=f"w_{name}_{half}", tag="w")
                src = wd[name][:] if half is None else wd[name][half]
                nc.sync.dma_start(t[:], src)
                return t

            def btile(name):
                t = ap.tile([P, KC], f32, name=f"b_{name}")
                nc.sync.dma_start(t[:], bd[name][:])
                return t

            # tiny throwaway AllReduce: absorbs the ~11.5us first-collective
            # stream-warmup cost that otherwise lands in the AllReduce-1 bubble
            dar_i = dp.tile([P, 1], f32, name="dar_i")
            dar_o = dp.tile([P, 1], f32, name="dar_o")
            nc.sync.dma_start(dar_i[:], bd["vq_b"][:, 0:1])
            nc.gpsimd.collective_compute(
                "AllReduce", ALU.add, replica_groups=GROUPS,
                ins=[dar_i[:]], outs=[dar_o[:]])

            # ---------------- Q projection first: minimal-dependency PE work
            b_vq = btile("vq_b")
            xT = ap.tile([P, KC, RV], bf16, name="xT")
            nc.sync.dma_start(xT[:, 0:3, :], xT_d[:, 0:3, :])
            w_vq = wp.tile([P, KC, H], bf16, name="w_vq", tag="w")
            nc.sync.dma_start(w_vq[:, 0:3, :], wd["vq_w"][:, 0:3, :])
            nc.sync.dma_start(xT[:, 3:6, :], xT_d[:, 3:6, :])
            nc.sync.dma_start(w_vq[:, 3:6, :], wd["vq_w"][:, 3:6, :])
            yT = ap.tile([P, KC, S], bf16, name="yT")
            nc.sync.dma_start(yT[:, :, 0:512], yT_d[:, :, 0:512])
            qT = ap.tile([P, KC, RV], bf16, name="qT")
            for wave in range(2):
                mcs = [3 * wave, 3 * wave + 1, 3 * wave + 2]
                pss = [pp.tile([P, RV], f32, name=f"qps{mc}", tag="sps",
                               bufs=3) for mc in mcs]
                for kc in range(KC):
                    for ps, mc in zip(pss, mcs):
                        nc.tensor.matmul(ps[:],
                                         w_vq[:, kc, mc * P:(mc + 1) * P],
                                         xT[:, kc, :],
                                         start=(kc == 0), stop=(kc == KC - 1))
                for ps, mc in zip(pss, mcs):
                    nc.scalar.activation(qT[:, mc, :], ps[:], AF.Identity,
                                         bias=b_vq[:, mc:mc + 1])

            b_vk = btile("vk_b")
            b_dv = btile("dv_b")


            # ---------------- K projection (full batch, channel-major) ------
            w_vk = wtile("vk_w")
            nc.sync.dma_start(yT[:, :, 512:S], yT_d[:, :, 512:S])
            kT = ap.tile([P, KC, S], bf16, name="kT")
            for mc in range(KC):
                for nh in range(2):
                    ps = pp.tile([P, 512], f32, name=f"kps{mc}_{nh}", tag="big",
                                 bufs=2)
                    for kc in range(KC):
                        nc.tensor.matmul(
                            ps[:], w_vk[:, kc, mc * P:(mc + 1) * P],
                            yT[:, kc, nh * 512:(nh + 1) * 512],
                            start=(kc == 0), stop=(kc == KC - 1))
                    nc.scalar.activation(kT[:, mc, nh * 512:(nh + 1) * 512], ps[:],
                                         AF.Identity, bias=b_vk[:, mc:mc + 1])

            # ---------------- V projection (row-major + ones col) -----------
            w_vv = wtile("vv_w")
            v_aug = ap.tile([P, JC, NH, DH + 1], bf16, name="v_aug")
            nc.vector.memset(v_aug[:, :, :, DH:DH + 1], 1.0)
            for jc in range(JC):
                for cg in range(2):
                    ps = pp.tile([P, 384], f32, name=f"vps{jc}_{cg}", tag="big",
                                 bufs=2)
                    for kc in range(KC):
                        nc.tensor.matmul(
                            ps[:], yT[:, kc, jc * P:(jc + 1) * P],
                            w_vv[:, kc, cg * 384:(cg + 1) * 384],
                            start=(kc == 0), stop=(kc == KC - 1))
                    nc.vector.tensor_copy(
                        v_aug[:, jc, cg * 6:(cg + 1) * 6, 0:DH],
                        ps[:].rearrange("p (h d) -> p h d", d=DH))

            # ---------------- diff-branch constants (per batch) -------------
            # m = mean_s(y) @ dv_w + dv_b ; theta1 = tanh(m @ WD_w)
            # bias1 = theta1 @ d_theta_w[:H] + d_theta_b
            # bias2 = m @ diff_out_w[:H] + diff_out_b   (computed late, in the
            # AllReduce-1 bubble).  Each matvec runs with the weight moving
            # (12 N=384 matmuls), then a 6-block PE transpose back to [P, KC].
            yb = ap.tile([P, KC], f32, name="yb")
            ybt = ap.tile([P, KC], bf16, name="ybt")
            for kc in range(KC):
                nc.vector.tensor_reduce(yb[:, kc:kc + 1], yT[:, kc, :],
                                        axis=mybir.AxisListType.X, op=ALU.add)
            nc.vector.tensor_scalar_mul(ybt[:], yb[:], 1.0 / S)

            def matvec(w_t, vec_t, func, name):
                """[1,768] row = func(vec_t^T @ w_t); transposed to [128, 6]
                via a DRAM round-trip (no PE-transpose ldweights).
                vec_t: [P, KC] bf16."""
                row = lp.tile([1, H], f32, name=f"{name}row", tag="mvrow")
                for cg in range(2):
                    ps = pp.tile([1, 384], f32, name=f"{name}ps{cg}", tag="small",
                                 bufs=1)
                    for kc in range(KC):
                        nc.tensor.matmul(ps[:], vec_t[:, kc:kc + 1],
                                         w_t[:, kc, cg * 384:(cg + 1) * 384],
                                         start=(kc == 0), stop=(kc == KC - 1))
                    nc.scalar.activation(row[:, cg * 384:(cg + 1) * 384], ps[:],
                                         func)
                rd = dp.tile([1, H], f32, name=f"{name}rd")
                nc.sync.dma_start(rd[:], row[:])
                psT = lp.tile([P, KC], f32, name=f"{name}T", tag="mvT")
                nc.sync.dma_start(
                    psT[:], rd.rearrange("o (kc p) -> p kc o", p=P)[:, :, 0])
                return psT

            w_dv = wtile("dv_w")
            mT = matvec(w_dv, ybt, AF.Identity, "m")
            m32 = ap.tile([P, KC], f32, name="m32")
            nc.vector.tensor_add(m32[:], mT[:], b_dv[:])
            mbf = ap.tile([P, KC], bf16, name="mbf")
            nc.vector.tensor_copy(mbf[:], m32[:])
            w_WD = wtile("WD_w")
            thT = matvec(w_WD, mbf, AF.Tanh, "th")
            th1 = ap.tile([P, KC], bf16, name="th1")
            nc.vector.tensor_copy(th1[:], thT[:])
            w_dth0 = wtile("d_theta_w", half=0)
            b_dth = btile("d_theta_b")
            b1T = matvec(w_dth0, th1, AF.Identity, "b1")
            bias1 = ap.tile([P, KC], f32, name="bias1")
            nc.vector.tensor_add(bias1[:], b1T[:], b_dth[:])

            # ---------------- attention (12 heads, 256 own queries) ---------
            if has_vvb:
                b_vv = btile("vv_b")
            vanT = ap.tile([P, KC, RV], bf16, name="vanT")

            def head_tail(h, pv):
                hc, ho = h // 2, (h % 2) * 64
                zsb = lp.tile([1, RV], f32, name=f"zsb{h}", tag="zsb")
                nc.vector.tensor_copy(zsb[:], pv[DH:DH + 1, :])
                invZ = lp.tile([1, RV], f32, name=f"invZ{h}", tag="invZ")
                nc.vector.reciprocal_approx_fast(invZ[:], zsb[:])
                bcs = lp.tile([64, RV], f32, name=f"bcs{h}", tag="bcs")
                nc.gpsimd.partition_broadcast(bcs[:], invZ[:], channels=64)
                nc.vector.tensor_mul(vanT[ho:ho + 64, hc, :], pv[0:DH, :], bcs[:])
                if has_vvb:
                    nc.vector.tensor_scalar_add(vanT[ho:ho + 64, hc, :],
                                                vanT[ho:ho + 64, hc, :],
                                                b_vv[ho:ho + 64, hc:hc + 1])

            for hp in range(NH // 2):
                h0, h1 = 2 * hp, 2 * hp + 1
                hc = hp
                e0 = lp.tile([P, JC, RV], bf16, name=f"expT{h0}", tag="expT",
                             bufs=4)
                e1h = lp.tile([P, JC, RV], bf16, name=f"expT{h1}", tag="expT",
                              bufs=4)
                pv0 = pp.tile([DH + 1, RV], f32, name=f"pv{h0}", tag="pv", bufs=2)
                pv1 = pp.tile([DH + 1, RV], f32, name=f"pv{h1}", tag="pv", bufs=2)
                for j2 in range(JC // 2):
                    ja, jb = 2 * j2, 2 * j2 + 1
                    for (h, ex) in ((h0, e0), (h1, e1h)):
                        ho = (h % 2) * 64
                        sps = pp.tile([P, 512], f32, name=f"sps{h}_{j2}",
                                      tag="big", bufs=2)
                        nc.tensor.matmul(sps[:, 0:RV],
                                         kT[ho:ho + 64, hc, ja * P:(ja + 1) * P],
                                         qT[ho:ho + 64, hc, :],
                                         start=True, stop=True)
                        nc.tensor.matmul(sps[:, RV:2 * RV],
                                         kT[ho:ho + 64, hc, jb * P:(jb + 1) * P],
                                         qT[ho:ho + 64, hc, :],
                                         start=True, stop=True)
                        nc.scalar.activation(
                            ex[:, ja:jb + 1, :].rearrange("p a b -> p (a b)"),
                            sps[:], AF.Exp, scale=SCALE)
                for jc in range(JC):
                    nc.tensor.matmul(pv0[:], v_aug[:, jc, h0, :], e0[:, jc, :],
                                     start=(jc == 0), stop=(jc == JC - 1))
                    nc.tensor.matmul(pv1[:], v_aug[:, jc, h1, :], e1h[:, jc, :],
                                     start=(jc == 0), stop=(jc == JC - 1))
                head_tail(h0, pv0)
                head_tail(h1, pv1)

            # ---------------- gating network ---------------------------------
            def gemm(pairs, func, bias_t=None, accum_t=None, name="g",
                     out_dt=bf16, pre=None):
                out = ap.tile([P, KC, RV], out_dt, name=name)
                nmm = len(pairs) * KC
                for wave in range(2):
                    mcs = [3 * wave, 3 * wave + 1, 3 * wave + 2]
                    pss = [pp.tile([P, RV], f32, name=f"{name}ps{mc}", tag="sps",
                                   bufs=3) for mc in mcs]
                    i = 0
                    for wt, at in pairs:
                        for kc in range(KC):
                            for ps, mc in zip(pss, mcs):
                                nc.tensor.matmul(
                                    ps[:], wt[:, kc, mc * P:(mc + 1) * P],
                                    at[:, kc, :],
                                    start=(i == 0), stop=(i == nmm - 1))
                            i += 1
                    for ps, mc in zip(pss, mcs):
                        src = ps
                        if pre is not None:
                            tmp = lp.tile([P, RV], f32, name=f"{name}pre{mc}",
                                          tag="pretmp")
                            nc.vector.tensor_add(tmp[:], ps[:], pre[:, mc, :])
                            src = tmp
                        nc.scalar.activation(
                            out[:, mc, :], src[:], func,
                            bias=(bias_t[:, mc:mc + 1]
                                  if bias_t is not None else 0.0),
                            accum_out=(accum_t[:, mc:mc + 1]
                                       if accum_t is not None else None))
                return out

            def allreduce6(part, name):
                ci = dp.tile([P, KC], f32, name=f"ci_{name}")
                co = dp.tile([P, KC], f32, name=f"co_{name}")
                nc.sync.dma_start(ci[:], part[:])
                nc.gpsimd.collective_compute(
                    "AllReduce", ALU.add, replica_groups=GROUPS,
                    ins=[ci[:]], outs=[co[:]])
                z = ap.tile([P, KC], f32, name=f"z_{name}")
                nc.sync.dma_start(z[:], co[:])
                return z

            def vec_w(wname):
                wt = wsp.tile([P, 2 * KC], bf16, name=f"ws_{wname}", tag="ws")
                nc.sync.dma_start(wt[:], wd[wname][:])
                return wt

            def vec_half(wt, base, at, ps):
                for kc in range(KC):
                    nc.tensor.matmul(ps[:], wt[:, base + kc:base + kc + 1],
                                     at[:, kc, :],
                                     start=(kc == 0), stop=(kc == KC - 1))

            w_vfc = wtile("van_fc_w")
            b_vfc = btile("van_fc_b")
            theta2 = gemm([(w_vfc, vanT)], AF.Tanh, bias_t=b_vfc, name="theta2")

            w_dth1 = wtile("d_theta_w", half=1)
            part1 = ap.tile([P, KC], f32, name="part1")
            e1 = gemm([(w_dth1, theta2)], AF.Exp, bias_t=bias1, accum_t=part1,
                      name="e1")
            z1 = allreduce6(part1, "z1")

            # --- AllReduce-1 bubble fillers (independent of z1) -------------
            w_WV = wtile("WV_w")
            gamma1 = gemm([(w_WV, vanT)], AF.Tanh, name="gamma1")
            w_vg0 = wtile("v_gamma_w", half=0)
            b_vg = btile("v_gamma_b")
            z2a = gemm([(w_vg0, gamma1)], AF.Identity, bias_t=b_vg, name="z2a",
                       out_dt=f32)
            w_vo0 = wtile("van_out_w", half=0)
            b_vo = btile("van_out_b")
            voa = gemm([(w_vo0, vanT)], AF.Identity, bias_t=b_vo, name="voa",
                       out_dt=f32)
            # bias2 matvec (needed only after z1, by dout)
            w_dout0 = wtile("diff_out_w", half=0)
            b_dout = btile("diff_out_b")
            b2T = matvec(w_dout0, mbf, AF.Identity, "b2")
            bias2 = ap.tile([P, KC], f32, name="bias2")
            nc.vector.tensor_add(bias2[:], b2T[:], b_dout[:])
            # vanT half of the nf output gate
            nw = vec_w("nf_out_w")
            ps_nfa = pp.tile([1, RV], f32, name="ps_nfa", tag="small", bufs=1)
            vec_half(nw, 0, vanT, ps_nfa)
            nfva = ap.tile([1, RV], f32, name="nfva")
            nc.scalar.activation(nfva[:], ps_nfa[:], AF.Identity)

            s1 = ap.tile([P, KC], f32, name="s1")
            nc.vector.reciprocal_approx_fast(s1[:], z1[:])
            nc.vector.tensor_mul(s1[:], s1[:], m32[:])
            dth = ap.tile([P, KC, RV], bf16, name="dth")
            for mc in range(KC):
                nc.vector.tensor_scalar_mul(dth[:, mc, :], e1[:, mc, :],
                                            s1[:, mc:mc + 1])

            w_dfc = wtile("diff_fc_w")
            b_dfc = btile("diff_fc_b")
            gamma2 = gemm([(w_dfc, dth)], AF.Tanh, bias_t=b_dfc, name="gamma2")

            w_vg1 = wtile("v_gamma_w", half=1)
            part2 = ap.tile([P, KC], f32, name="part2")
            e2 = gemm([(w_vg1, gamma2)], AF.Exp, accum_t=part2, pre=z2a,
                      name="e2")
            z2 = allreduce6(part2, "z2")

            # --- AllReduce-2 bubble fillers (diff branch tail) --------------
            w_dout1 = wtile("diff_out_w", half=1)
            dout = gemm([(w_dout1, dth)], AF.Tanh, bias_t=bias2, name="dout")
            w_dfus = wtile("diff_fus_w")
            b_dfus = btile("diff_fus_b")
            dfus = gemm([(w_dfus, dout)], AF.Tanh, bias_t=b_dfus, name="dfus")
            gw = vec_w("gate_w")
            ps_ga = pp.tile([1, RV], f32, name="ps_ga", tag="small", bufs=1)
            vec_half(gw, 0, dfus, ps_ga)
            ga = ap.tile([1, RV], f32, name="ga")
            nc.scalar.activation(ga[:], ps_ga[:], AF.Identity)

            s2 = ap.tile([P, KC], f32, name="s2")
            nc.vector.reciprocal_approx_fast(s2[:], z2[:])
            ag = ap.tile([P, KC, RV], bf16, name="ag")
            for mc in range(KC):
                nc.vector.scalar_tensor_tensor(
                    ag[:, mc, :], e2[:, mc, :], s2[:, mc:mc + 1],
                    vanT[:, mc, :], op0=ALU.mult, op1=ALU.mult)

            w_vo1 = wtile("van_out_w", half=1)
            vout = gemm([(w_vo1, ag)], AF.Tanh, pre=voa, name="vout")
            w_vfus = wtile("van_fus_w")
            b_vfus = btile("van_fus_b")
            vfus = gemm([(w_vfus, vout)], AF.Tanh, bias_t=b_vfus, name="vfus")

            # gate: add the vfus half, sigmoid, broadcast
            ps_gb = pp.tile([1, RV], f32, name="ps_gb", tag="small", bufs=1)
            vec_half(gw, KC, vfus, ps_gb)
            gsum = lp.tile([1, RV], f32, name="gsum", tag="gsum")
            nc.vector.tensor_add(gsum[:], ps_gb[:], ga[:])
            g = ap.tile([1, RV], f32, name="g")
            nc.scalar.activation(g[:], gsum[:], AF.Sigmoid)
            gbc = ap.tile([P, RV], f32, name="gbc")
            nc.gpsimd.partition_broadcast(gbc[:], g[:])

            fus = ap.tile([P, KC, RV], bf16, name="fus")
            for mc in range(KC):
                eng = nc.gpsimd if mc in (1, 4) else nc.vector
                t1 = lp.tile([P, RV], bf16, name=f"ft1_{mc}", tag="ft1")
                eng.tensor_sub(t1[:], vfus[:, mc, :], dfus[:, mc, :])
                t2 = lp.tile([P, RV], bf16, name=f"ft2_{mc}", tag="ft2")
                eng.tensor_mul(t2[:], t1[:], gbc[:])
                eng.tensor_add(fus[:, mc, :], t2[:], dfus[:, mc, :])

            w_nf = wtile("nf_w")
            b_nf = btile("nf_b")
            w_fin = wtile("final_w")
            b_fin = btile("final_b")
            tnf = gemm([(w_nf, fus)], AF.Identity, bias_t=b_nf, name="tnf")
            ft = gemm([(w_fin, fus)], AF.Tanh, bias_t=b_fin, name="ftanh")
            ps_nfb = pp.tile([1, RV], f32, name="ps_nfb", tag="small", bufs=1)
            vec_half(nw, KC, tnf, ps_nfb)
            nfsum = lp.tile([1, RV], f32, name="nfsum", tag="gsum")
            nc.vector.tensor_add(nfsum[:], ps_nfb[:], nfva[:])
            nfv = ap.tile([1, RV], f32, name="nfv")
            nc.scalar.activation(nfv[:], nfsum[:], AF.Sigmoid)
            nbc = ap.tile([P, RV], f32, name="nbc")
            nc.gpsimd.partition_broadcast(nbc[:], nfv[:])

            outT = ap.tile([P, KC, RV], f32, name="outT")
            for mc in range(KC):
                eng = nc.gpsimd if mc in (1, 4) else nc.vector
                eng.tensor_mul(outT[:, mc, :], ft[:, mc, :], nbc[:])
                nc.sync.dma_start(out_d[:, mc, :], outT[:, mc, :])

    nc.compile()
    return nc


def _pack_w(w):
    """[768, N] -> [P, KC, N] with row kc*128+p on partition p."""
    return np.ascontiguousarray(
        w.reshape(KC, P, -1).transpose(1, 0, 2)).astype(nbf16)


def _pack_b(b):
    return np.ascontiguousarray(b.reshape(KC, P).T.astype(np.float32))


def make_in_maps(inputs):
    x = np.asarray(inputs["x"], np.float32)
    y = np.asarray(inputs["y"], np.float32)
    has_vvb = bool(np.any(np.asarray(inputs["vv_b"]) != 0))

    base = {}
    for w in W768:
        base[w] = _pack_w(np.asarray(inputs[w], np.float32))
    for w in W1536:
        wf = np.asarray(inputs[w], np.float32)
        base[w] = np.ascontiguousarray(
            np.stack([_pack_w(wf[:H]), _pack_w(wf[H:])]))
    for w in ["gate_w", "nf_out_w"]:
        wf = np.asarray(inputs[w], np.float32)[:, 0]
        base[w] = np.ascontiguousarray(wf.reshape(2 * KC, P).T.astype(nbf16))
    for b in BIAS:
        base[b] = _pack_b(np.asarray(inputs[b], np.float32))
    if has_vvb:
        base["vv_b"] = _pack_b(np.asarray(inputs["vv_b"], np.float32))

    xt = x.reshape(B * S, H).T  # [H, 2048]
    in_maps = []
    for c in range(8):
        m = dict(base)
        m["xT"] = _pack_w(xt[:, c * RV:(c + 1) * RV])
        m["yT"] = _pack_w(y[c // 4].T)
        in_maps.append(m)
    return in_maps, has_vvb


_CACHE = {}


def kernel(**inputs):
    in_maps, has_vvb = make_in_maps(inputs)
    if has_vvb not in _CACHE:
        _CACHE[has_vvb] = build(has_vvb)
    nc = _CACHE[has_vvb]

    res = run_bass_kernel_spmd(nc, in_maps, core_ids=list(range(8)))
    # outT per core: [P, KC, RV] with channel kc*128+p at [p, kc]
    cols = [np.asarray(res.results[c]["outT"], np.float32)
            .transpose(1, 0, 2).reshape(H, RV) for c in range(8)]
    full = np.concatenate(cols, axis=1)  # [H, 2048]
    return np.ascontiguousarray(full.T.reshape(B, S, H)).astype(np.float32)


if __name__ == "__main__":
    rng = np.random.default_rng(0)
    ins = {"x": rng.standard_normal((B, S, H)).astype(np.float32),
           "y": rng.standard_normal((B, S, H)).astype(np.float32)}
    for w in W768 + W1536:
        shp = (H, H) if w in W768 else (2 * H, H)
        ins[w] = (rng.standard_normal(shp) * 0.02).astype(np.float32)
    ins["gate_w"] = (rng.standard_normal((2 * H, 1)) * 0.02).astype(np.float32)
    ins["nf_out_w"] = (rng.standard_normal((2 * H, 1)) * 0.02).astype(np.float32)
    for b in BIAS + ["vv_b"]:
        ins[b] = np.zeros(H, np.float32)
    out = kernel(**ins)
    print("out", out.shape, out.dtype, np.abs(out).mean())
